# revision 1
# baseline (speedup 1.0000x reference)
"""Trainium2 Bass kernel for nn_ContrastiveLoss (NT-Xent style contrastive loss).

Strategy (8 NeuronCores, SPMD):
  - Host sorts samples by label (the scalar loss is permutation invariant),
    row-normalizes, and builds X^T [D=128, N=8192] in bf16.
  - Rows are sharded across 8 cores (1024 rows each, 8 blocks of 128).
  - Each core computes its [1024, 8192] similarity block against the full
    X^T (the "all-gathered" copy arrives as a per-core input), reduces
    exp-row-sums on-chip, and evaluates the positive-pair terms only on a
    narrow label-band window (sorted labels make positives contiguous).
  - Per-row partial losses return to the host, which sums them and divides
    by the exact positive-pair count (from the label histogram).

Math: with e_ij = exp(sim_ij/T), S_i = sum_j e_ij (incl diag),
P_i = sum_{j in label-range(i)} e_ij (incl diag), unsim_i = S_i - P_i,
u_i = log(unsim_i), the reference loss row-sum equals
  npos_i*u_i + sum_{range} softplus(sim_ij/T - u_i) - softplus(1/T - u_i)
             - (sum_{range} sim_ij/T - 1/T)
where npos_i = (label count of i) - 1. The diagonal contributions cancel
exactly in unsim and are removed via the constant sim_ii = 1 (rows are
normalized; the fp difference is ~1e-9 relative on the final scalar).
"""

import numpy as np

T = 0.2
INV_T = 1.0 / T  # 5.0
EPS = 1e-5
N, D, NCLASS = 8192, 128, 128
NCORES = 8
ROWS_PER_CORE = N // NCORES          # 1024
BLOCKS = ROWS_PER_CORE // 128        # 8 blocks of 128 rows per core
CHUNK = 2048                         # ACT chunk (4 PSUM banks)
NCHUNKS = N // CHUNK                 # 4 per block
MM = 512                             # matmul free-dim per PSUM bank

_CACHE = {}


def _build_nc(W, debug=False):
    """Build the SPMD Bass/Tile program. W = band window width (mult of 512)."""
    import concourse.bass as bass
    import concourse.bacc as bacc
    import concourse.mybir as mybir
    import concourse.tile as tile

    dt = mybir.dt
    AF = mybir.ActivationFunctionType
    ALU = mybir.AluOpType
    X = mybir.AxisListType.X

    nc = bacc.Bacc("TRN2", target_bir_lowering=False, debug=debug)

    xt_d = nc.dram_tensor("xt", [128, N], dt.bfloat16, kind="ExternalInput")
    xtown_d = nc.dram_tensor("xtown", [128, ROWS_PER_CORE], dt.bfloat16,
                             kind="ExternalInput")
    xtband_d = nc.dram_tensor("xtband", [128, BLOCKS * W], dt.bfloat16,
                              kind="ExternalInput")
    gsr_d = nc.dram_tensor("gsr", [128, BLOCKS], dt.float32, kind="ExternalInput")
    ger_d = nc.dram_tensor("ger", [128, BLOCKS], dt.float32, kind="ExternalInput")
    npos_d = nc.dram_tensor("npos", [128, BLOCKS], dt.float32, kind="ExternalInput")
    out_d = nc.dram_tensor("out", [128, BLOCKS], dt.float32, kind="ExternalOutput")

    nwc = W // MM  # band matmul sub-chunks

    with tile.TileContext(nc) as tc:
        with (
            tc.tile_pool(name="const", bufs=1) as const,
            tc.tile_pool(name="band", bufs=1) as band,
            tc.tile_pool(name="etmp", bufs=3) as etmp_pool,
            tc.tile_pool(name="sp", bufs=2) as sp_pool,
            tc.tile_pool(name="small", bufs=1) as small,
            tc.tile_pool(name="psum", bufs=2, space="PSUM") as psum,
        ):
            # ---- persistent loads ----
            xt = const.tile([128, N], dt.bfloat16)
            for k in range(N // CHUNK):
                nc.sync.dma_start(xt[:, k * CHUNK:(k + 1) * CHUNK],
                                  xt_d[:, k * CHUNK:(k + 1) * CHUNK])
            xtown = const.tile([128, ROWS_PER_CORE], dt.bfloat16)
            nc.sync.dma_start(xtown[:], xtown_d[:])
            xtband = const.tile([128, BLOCKS * W], dt.bfloat16)
            nc.sync.dma_start(xtband[:], xtband_d[:])
            gsr = const.tile([128, BLOCKS], dt.float32)
            nc.sync.dma_start(gsr[:], gsr_d[:])
            ger = const.tile([128, BLOCKS], dt.float32)
            nc.sync.dma_start(ger[:], ger_d[:])
            npos = const.tile([128, BLOCKS], dt.float32)
            nc.sync.dma_start(npos[:], npos_d[:])

            iota_i = const.tile([128, W], dt.int32)
            nc.gpsimd.iota(iota_i[:], pattern=[[1, W]], base=0, channel_multiplier=0)
            iota_f = const.tile([128, W], dt.float32)
            nc.vector.tensor_copy(iota_f[:], iota_i[:])

            acc = const.tile([128, BLOCKS], dt.float32)

            # per-block persistent tiles
            s_band = [band.tile([128, W], dt.float32, name=f"sb{b}") for b in range(BLOCKS)]
            e_band = [band.tile([128, W], dt.float32, name=f"eb{b}") for b in range(BLOCKS)]
            mask = [band.tile([128, W], dt.float32, name=f"mk{b}") for b in range(BLOCKS)]
            S = [small.tile([128, 1], dt.float32, name=f"S{b}") for b in range(BLOCKS)]
            P = [small.tile([128, 1], dt.float32, name=f"P{b}") for b in range(BLOCKS)]
            u = [small.tile([128, 1], dt.float32, name=f"u{b}") for b in range(BLOCKS)]
            runsim = [small.tile([128, 1], dt.float32, name=f"ru{b}") for b in range(BLOCKS)]
            spd = [small.tile([128, 1], dt.float32, name=f"sd{b}") for b in range(BLOCKS)]
            sparts = [small.tile([128, NCHUNKS], dt.float32, name=f"sp{b}")
                      for b in range(BLOCKS)]

            # ---- Phase A: dense exp row-sums (Exp table) + band sims ----
            for b in range(BLOCKS):
                lhsT = xtown[:, b * 128:(b + 1) * 128]
                for kc in range(NCHUNKS):
                    ps = psum.tile([128, CHUNK], dt.float32, tag="ps")
                    for j in range(CHUNK // MM):
                        c0 = kc * CHUNK + j * MM
                        nc.tensor.matmul(ps[:, j * MM:(j + 1) * MM], lhsT,
                                         xt[:, c0:c0 + MM], start=True, stop=True)
                    e_tmp = etmp_pool.tile([128, CHUNK], dt.float32, tag="et")
                    nc.scalar.activation(e_tmp[:], ps[:], AF.Exp, bias=0.0,
                                         scale=INV_T,
                                         accum_out=sparts[b][:, kc:kc + 1])
                # band: sims for the W-wide positive window
                psb = psum.tile([128, W], dt.float32, tag="ps")
                for j in range(nwc):
                    nc.tensor.matmul(psb[:, j * MM:(j + 1) * MM], lhsT,
                                     xtband[:, b * W + j * MM: b * W + (j + 1) * MM],
                                     start=True, stop=True)
                nc.scalar.activation(e_band[b][:], psb[:], AF.Exp, bias=0.0,
                                     scale=INV_T)
                nc.vector.tensor_copy(s_band[b][:], psb[:])
                nc.vector.reduce_sum(S[b][:], sparts[b][:], axis=X)

            # ---- Phase B: range masks + positive-window sums (DVE only) ----
            tmp_pool = sp_pool
            for b in range(BLOCKS):
                m1 = tmp_pool.tile([128, W], dt.float32, tag="m1")
                nc.vector.tensor_scalar(m1[:], iota_f[:], gsr[:, b:b + 1], None,
                                        op0=ALU.is_ge)
                nc.vector.scalar_tensor_tensor(mask[b][:], iota_f[:],
                                               ger[:, b:b + 1], m1[:],
                                               op0=ALU.is_lt, op1=ALU.mult)
                ttmp = tmp_pool.tile([128, W], dt.float32, tag="tt")
                nc.vector.tensor_mul(ttmp[:], e_band[b][:], mask[b][:])
                nc.vector.reduce_sum(P[b][:], ttmp[:], axis=X)
                # unsim = S - P  (reuse P tile as unsim)
                nc.vector.tensor_sub(P[b][:], S[b][:], P[b][:])

            # ---- Phase C: u = log(unsim), runsim = 1/unsim ----
            # exp(sim/T - u) == e_band * runsim, so phase D needs no Exp at
            # all: the ACT stream is all-Exp (phase A) then all-Ln, keeping
            # one activation-table set loaded per phase (2 loads total).
            for b in range(BLOCKS):
                nc.scalar.activation(u[b][:], P[b][:], AF.Ln)
            for b in range(BLOCKS):
                nc.vector.reciprocal(runsim[b][:], P[b][:])

            # ---- Phase D: softplus terms via Ln(1 + e*runsim) (Ln table) ----
            E5 = float(np.exp(5.0))
            sp_tiles = []
            for b in range(BLOCKS):
                t2 = small.tile([128, 1], dt.float32, name=f"t2{b}")
                nc.vector.tensor_scalar_mul(t2[:], runsim[b][:], E5)
                nc.scalar.activation(spd[b][:], t2[:], AF.Ln, bias=1.0)
                et = sp_pool.tile([128, W], dt.float32, tag="spe")
                nc.vector.tensor_scalar(et[:], e_band[b][:], runsim[b][:], None,
                                        op0=ALU.mult)
                sp = sp_pool.tile([128, W], dt.float32, tag="spt")
                nc.scalar.activation(sp[:], et[:], AF.Ln, bias=1.0)
                sp_tiles.append(sp)

                # ---- Phase E interleaved (DVE): A, B, combine ----
                A = small.tile([128, 1], dt.float32, name=f"A{b}")
                B = small.tile([128, 1], dt.float32, name=f"B{b}")
                ttmp = tmp_pool.tile([128, W], dt.float32, tag="tt")
                nc.vector.tensor_mul(ttmp[:], sp[:], mask[b][:])
                nc.vector.reduce_sum(A[:], ttmp[:], axis=X)
                ttmp2 = tmp_pool.tile([128, W], dt.float32, tag="tt")
                nc.vector.tensor_mul(ttmp2[:], s_band[b][:], mask[b][:])
                nc.vector.reduce_sum(B[:], ttmp2[:], axis=X)
                # loss = npos*u + A - spd - (INV_T*B - INV_T)
                r1 = small.tile([128, 1], dt.float32, name=f"r1{b}")
                nc.vector.scalar_tensor_tensor(r1[:], u[b][:], npos[:, b:b + 1],
                                               A[:], op0=ALU.mult, op1=ALU.add)
                r2 = small.tile([128, 1], dt.float32, name=f"r2{b}")
                nc.vector.tensor_scalar(r2[:], B[:], INV_T, -INV_T,
                                        op0=ALU.mult, op1=ALU.add)
                r3 = small.tile([128, 1], dt.float32, name=f"r3{b}")
                nc.vector.tensor_add(r3[:], r2[:], spd[b][:])
                nc.vector.tensor_sub(acc[:, b:b + 1], r1[:], r3[:])

            nc.sync.dma_start(out_d[:], acc[:])

    nc.compile()
    return nc


def _prep(input, label):
    """Host-side shard prep: sort by label, normalize, build per-core inputs."""
    import ml_dtypes

    x = np.asarray(input, dtype=np.float32).reshape(N, D)
    lab = np.asarray(label).astype(np.int64).reshape(N)

    order = np.argsort(lab, kind="stable")
    xs, ls = x[order], lab[order]
    counts = np.bincount(ls, minlength=NCLASS)
    n_pos = int((counts.astype(np.int64) ** 2).sum()) - N
    ends = np.cumsum(counts)
    starts = ends - counts
    row_gs = starts[ls]          # [N] group start col per (sorted) row
    row_ge = ends[ls]            # [N] group end col per row

    norms = np.sqrt((xs * xs).sum(1, dtype=np.float32)).astype(np.float32)
    # reference divides by max(n_i*n_j, EPS); for this data the max never
    # binds (norms ~ 11), so plain normalization is exact.
    assert float(norms.min()) ** 2 > EPS * 1.0001
    xn = (xs / norms[:, None]).astype(np.float32)
    xt = np.ascontiguousarray(xn.T).astype(ml_dtypes.bfloat16)  # [128, N]

    # band windows per global block
    nblk = N // 128
    lo = row_gs[np.arange(nblk) * 128]
    hi = row_ge[np.arange(nblk) * 128 + 127]
    maxband = int((hi - lo).max())
    W = max(512, ((maxband + 511) // 512) * 512)
    wstart = np.minimum(lo, N - W)

    in_maps = []
    for c in range(NCORES):
        r0 = c * ROWS_PER_CORE
        xtband = np.empty((128, BLOCKS * W), dtype=ml_dtypes.bfloat16)
        gsr = np.empty((128, BLOCKS), np.float32)
        ger = np.empty((128, BLOCKS), np.float32)
        npos = np.empty((128, BLOCKS), np.float32)
        for b in range(BLOCKS):
            g = c * BLOCKS + b
            ws = int(wstart[g])
            xtband[:, b * W:(b + 1) * W] = xt[:, ws:ws + W]
            rows = slice(r0 + b * 128, r0 + (b + 1) * 128)
            gsr[:, b] = (row_gs[rows] - ws).astype(np.float32)
            ger[:, b] = (row_ge[rows] - ws).astype(np.float32)
            npos[:, b] = (row_ge[rows] - row_gs[rows] - 1).astype(np.float32)
        in_maps.append({
            "xt": xt,
            "xtown": np.ascontiguousarray(
                xt[:, r0:r0 + ROWS_PER_CORE]),
            "xtband": xtband,
            "gsr": gsr,
            "ger": ger,
            "npos": npos,
        })
    return in_maps, n_pos, W


def kernel(input, label):
    from concourse.bass_utils import run_bass_kernel_spmd

    in_maps, n_pos, W = _prep(input, label)
    if W not in _CACHE:
        _CACHE[W] = _build_nc(W)
    nc = _CACHE[W]

    res = None
    for attempt in range(4):
        try:
            res = run_bass_kernel_spmd(nc, in_maps, core_ids=list(range(NCORES)))
            break
        except Exception:
            if attempt == 3:
                raise
            import time
            time.sleep(45)  # device may need a moment to recover
    global LAST_RESULTS
    LAST_RESULTS = res
    total = 0.0
    for r in res.results:
        total += float(np.sum(r["out"], dtype=np.float64))
    return np.array(total / n_pos, dtype=np.float32)


LAST_RESULTS = None



# revision 2
# speedup vs baseline: 1.3134x; 1.3134x over previous
"""Trainium2 Bass kernel for nn_ContrastiveLoss (NT-Xent style contrastive loss).

Strategy (8 NeuronCores, SPMD):
  - Host sorts samples by label (the scalar loss is permutation invariant),
    row-normalizes, and builds X^T [D=128, N=8192] in bf16.
  - Rows are sharded across 8 cores (1024 rows each, 8 blocks of 128).
  - Each core computes its [1024, 8192] similarity block against the full
    X^T; the exp row-sums are split between the ACT engine (exact Exp with
    accum) and the DVE (fp16 Schraudolph bit-trick exp + reduce), balancing
    the two engine's throughputs. The positive-pair window (sorted labels
    make positives contiguous) is handled by a masked band: one fused DVE
    affine_mul_reduce gives both the masked sims (exp input) and their sum B;
    one ACT Exp-with-accum gives the masked exp sum.
  - Device outputs per-row partial sums only (no log/reciprocal on device:
    the ACT engine keeps a single Exp table load, no table thrash). The host
    computes the final scalar:
      loss_row = npos*log(unsim) + (P - e^5)/unsim - 5*B + 5
    using log1p(x) ~= x (x = e_ij/unsim <= 1.6e-3 here; error ~1e-7 rel).
  - The Schraudolph share's multiplicative bias is calibrated out on the
    host from a small sampled subset of the exact sims (divide by rbar).
"""

import numpy as np

T = 0.2
INV_T = 1.0 / T  # 5.0
EPS = 1e-5
N, D, NCLASS = 8192, 128, 128
NCORES = 8
ROWS_PER_CORE = N // NCORES          # 1024
BLOCKS = ROWS_PER_CORE // 128        # 8 blocks of 128 rows per core
CHUNK = 2048                         # PSUM chunk (4 banks)
NCHUNKS = N // CHUNK                 # 4 per block
MM = 512                             # matmul free-dim per PSUM bank
AW = 1248                            # ACT's columns per chunk (exact Exp)
DW = CHUNK - AW                      # DVE's columns per chunk (Schraudolph)

# fp16 Schraudolph: exp(5*s) ~= bitcast_i16_to_f16(round(s*SCH_A + SCH_B))
SCH_A = INV_T * 1024.0 * 1.4426950408889634   # 5 * 1024 * log2(e)
SCH_B = 15302.0                               # 15*1024 - 58 (mean-centering)

_CACHE = {}


def _build_nc(W, debug=False):
    """Build the SPMD Bass/Tile program. W = band window width (mult of 512)."""
    import concourse.bass as bass
    import concourse.bacc as bacc
    import concourse.mybir as mybir
    import concourse.tile as tile

    dt = mybir.dt
    AF = mybir.ActivationFunctionType
    ALU = mybir.AluOpType
    X = mybir.AxisListType.X

    nc = bacc.Bacc("TRN2", target_bir_lowering=False, debug=debug)

    xt_d = nc.dram_tensor("xt", [128, N], dt.bfloat16, kind="ExternalInput")
    xtown_d = nc.dram_tensor("xtown", [128, ROWS_PER_CORE], dt.bfloat16,
                             kind="ExternalInput")
    xtband_d = nc.dram_tensor("xtband", [128, BLOCKS * W], dt.bfloat16,
                              kind="ExternalInput")
    gsr_d = nc.dram_tensor("gsr", [128, BLOCKS], dt.float32, kind="ExternalInput")
    ger_d = nc.dram_tensor("ger", [128, BLOCKS], dt.float32, kind="ExternalInput")
    sp_d = nc.dram_tensor("sp", [128, BLOCKS * NCHUNKS], dt.float32,
                          kind="ExternalOutput")
    dp_d = nc.dram_tensor("dp", [128, BLOCKS * NCHUNKS], dt.float32,
                          kind="ExternalOutput")
    a1_d = nc.dram_tensor("a1", [128, BLOCKS], dt.float32, kind="ExternalOutput")
    bb_d = nc.dram_tensor("bb", [128, BLOCKS], dt.float32, kind="ExternalOutput")

    nwc = W // MM  # band matmul sub-chunks

    with tile.TileContext(nc) as tc:
        with (
            tc.tile_pool(name="const", bufs=1) as const,
            tc.tile_pool(name="escr", bufs=3) as escr_pool,
            tc.tile_pool(name="i16", bufs=3) as i16_pool,
            tc.tile_pool(name="ms", bufs=2) as ms_pool,
            tc.tile_pool(name="psum", bufs=2, space="PSUM") as psum,
        ):
            # ---- small inputs first (masks depend on them) ----
            gsr = const.tile([128, BLOCKS], dt.float32)
            nc.sync.dma_start(gsr[:], gsr_d[:])
            ger = const.tile([128, BLOCKS], dt.float32)
            nc.sync.dma_start(ger[:], ger_d[:])
            xtown = const.tile([128, ROWS_PER_CORE], dt.bfloat16)
            nc.sync.dma_start(xtown[:], xtown_d[:])
            xt = const.tile([128, N], dt.bfloat16)
            for k in range(N // 1024):
                nc.sync.dma_start(xt[:, k * 1024:(k + 1) * 1024],
                                  xt_d[:, k * 1024:(k + 1) * 1024])
            xtband = const.tile([128, BLOCKS * W], dt.bfloat16)
            nc.sync.dma_start(xtband[:], xtband_d[:])

            iota_i = const.tile([128, W], dt.int32)
            nc.gpsimd.iota(iota_i[:], pattern=[[1, W]], base=0,
                           channel_multiplier=0)
            iota_f = const.tile([128, W], dt.float32)
            nc.vector.tensor_copy(iota_f[:], iota_i[:])

            # ---- persistent accumulator outputs ----
            sparts = const.tile([128, BLOCKS * NCHUNKS], dt.float32)
            dparts = const.tile([128, BLOCKS * NCHUNKS], dt.float32)
            a1 = const.tile([128, BLOCKS], dt.float32)
            bb = const.tile([128, BLOCKS], dt.float32)
            masks = [const.tile([128, W], dt.float32, name=f"mk{b}")
                     for b in range(BLOCKS)]

            for b in range(BLOCKS):
                # band mask for this block (DVE, overlaps with dense below)
                m1 = ms_pool.tile([128, W], dt.float32, tag="m1")
                nc.vector.tensor_scalar(m1[:], iota_f[:], gsr[:, b:b + 1],
                                        None, op0=ALU.is_ge)
                nc.vector.scalar_tensor_tensor(masks[b][:], iota_f[:],
                                               ger[:, b:b + 1], m1[:],
                                               op0=ALU.is_lt, op1=ALU.mult)

                lhsT = xtown[:, b * 128:(b + 1) * 128]
                for kc in range(NCHUNKS):
                    ps = psum.tile([128, CHUNK], dt.float32, tag="ps")
                    for j in range(CHUNK // MM):
                        c0 = kc * CHUNK + j * MM
                        nc.tensor.matmul(ps[:, j * MM:(j + 1) * MM], lhsT,
                                         xt[:, c0:c0 + MM], start=True,
                                         stop=True)
                    col = b * NCHUNKS + kc
                    # ACT: exact exp + row-sum accumulate on [0:AW]
                    e_scr = escr_pool.tile([128, AW], dt.bfloat16, tag="et")
                    nc.scalar.activation(e_scr[:], ps[:, 0:AW], AF.Exp,
                                         bias=0.0, scale=INV_T,
                                         accum_out=sparts[:, col:col + 1])
                    # DVE: Schraudolph fp16 exp + reduce on [AW:CHUNK]
                    i16 = i16_pool.tile([128, DW], dt.int16, tag="i16")
                    nc.vector.tensor_scalar(i16[:], ps[:, AW:CHUNK],
                                            SCH_A, SCH_B,
                                            op0=ALU.mult, op1=ALU.add)
                    nc.vector.reduce_sum(dparts[:, col:col + 1],
                                         i16[:].bitcast(dt.float16), axis=X)

                # band: masked sims (one fused DVE op -> ms and B), then
                # one ACT exp with accum -> masked exp sum (+ W-count ones)
                psb = psum.tile([128, W], dt.float32, tag="ps")
                for j in range(nwc):
                    nc.tensor.matmul(psb[:, j * MM:(j + 1) * MM], lhsT,
                                     xtband[:, b * W + j * MM:
                                            b * W + (j + 1) * MM],
                                     start=True, stop=True)
                ms = ms_pool.tile([128, W], dt.float32, tag="ms")
                nc.vector.affine_mul_reduce(ms[:], bb[:, b:b + 1],
                                            psb[:], masks[b][:], 1.0, 0.0)
                eb = escr_pool.tile([128, W], dt.bfloat16, tag="eb")
                nc.scalar.activation(eb[:], ms[:], AF.Exp, bias=0.0,
                                     scale=INV_T,
                                     accum_out=a1[:, b:b + 1])

            nc.sync.dma_start(sp_d[:], sparts[:])
            nc.sync.dma_start(dp_d[:], dparts[:])
            nc.sync.dma_start(a1_d[:], a1[:])
            nc.sync.dma_start(bb_d[:], bb[:])

    nc.compile()
    return nc


def _prep(input, label):
    """Host-side shard prep: sort by label, normalize, build per-core inputs."""
    import ml_dtypes

    x = np.asarray(input, dtype=np.float32).reshape(N, D)
    lab = np.asarray(label).astype(np.int64).reshape(N)

    order = np.argsort(lab, kind="stable")
    xs, ls = x[order], lab[order]
    counts = np.bincount(ls, minlength=NCLASS)
    n_pos = int((counts.astype(np.int64) ** 2).sum()) - N
    ends = np.cumsum(counts)
    starts = ends - counts
    row_gs = starts[ls]          # [N] group start col per (sorted) row
    row_ge = ends[ls]            # [N] group end col per row

    norms = np.sqrt((xs * xs).sum(1, dtype=np.float32)).astype(np.float32)
    # reference divides by max(n_i*n_j, EPS); for this data the max never
    # binds (norms ~ 11), so plain normalization is exact.
    assert float(norms.min()) ** 2 > EPS * 1.0001
    xn = (xs / norms[:, None]).astype(np.float32)
    xt = np.ascontiguousarray(xn.T).astype(ml_dtypes.bfloat16)  # [128, N]

    # Schraudolph bias calibration on a sampled subset of exact sims
    xnb = xt.T.astype(np.float32)                       # bf16-rounded rows
    samp = xnb[:: N // 48] @ xnb.T                      # [~48, N] sims
    ys = np.float64(INV_T) * samp.astype(np.float64)
    i16 = np.rint(samp * SCH_A + SCH_B).astype(np.int16)
    rbar = float(i16.view(np.float16).astype(np.float64).sum()
                 / np.exp(ys).sum())

    # band windows per global block
    nblk = N // 128
    lo = row_gs[np.arange(nblk) * 128]
    hi = row_ge[np.arange(nblk) * 128 + 127]
    maxband = int((hi - lo).max())
    W = max(512, ((maxband + 511) // 512) * 512)
    wstart = np.minimum(lo, N - W)

    in_maps = []
    for c in range(NCORES):
        r0 = c * ROWS_PER_CORE
        xtband = np.empty((128, BLOCKS * W), dtype=ml_dtypes.bfloat16)
        gsr = np.empty((128, BLOCKS), np.float32)
        ger = np.empty((128, BLOCKS), np.float32)
        for b in range(BLOCKS):
            g = c * BLOCKS + b
            ws = int(wstart[g])
            xtband[:, b * W:(b + 1) * W] = xt[:, ws:ws + W]
            rows = slice(r0 + b * 128, r0 + (b + 1) * 128)
            gsr[:, b] = (row_gs[rows] - ws).astype(np.float32)
            ger[:, b] = (row_ge[rows] - ws).astype(np.float32)
        in_maps.append({
            "xt": xt,
            "xtown": np.ascontiguousarray(xt[:, r0:r0 + ROWS_PER_CORE]),
            "xtband": xtband,
            "gsr": gsr,
            "ger": ger,
        })
    counts_row = (row_ge - row_gs).astype(np.float64)   # class size per row
    return in_maps, n_pos, W, counts_row, rbar


def kernel(input, label):
    from concourse.bass_utils import run_bass_kernel_spmd

    in_maps, n_pos, W, counts_row, rbar = _prep(input, label)
    if W not in _CACHE:
        _CACHE[W] = _build_nc(W)
    nc = _CACHE[W]

    res = None
    for attempt in range(4):
        try:
            res = run_bass_kernel_spmd(nc, in_maps, core_ids=list(range(NCORES)))
            break
        except Exception:
            if attempt == 3:
                raise
            import time
            time.sleep(45)  # device may need a moment to recover
    global LAST_RESULTS
    LAST_RESULTS = res

    E5 = float(np.exp(5.0))
    total = 0.0
    for c, r in enumerate(res.results):
        sp = np.asarray(r["sp"], dtype=np.float64)      # [128, BLOCKS*NCHUNKS]
        dp = np.asarray(r["dp"], dtype=np.float64)
        a1 = np.asarray(r["a1"], dtype=np.float64)      # [128, BLOCKS]
        bbv = np.asarray(r["bb"], dtype=np.float64)
        # S per row: exact ACT part + bias-corrected Schraudolph part
        S = (sp.reshape(128, BLOCKS, NCHUNKS).sum(axis=2)
             + dp.reshape(128, BLOCKS, NCHUNKS).sum(axis=2) / rbar)  # [128, B]
        cnt = counts_row[c * ROWS_PER_CORE:(c + 1) * ROWS_PER_CORE]
        cnt = cnt.reshape(BLOCKS, 128).T                # [128, BLOCKS]
        P = a1 - (W - cnt)                              # masked exp sum w/ diag
        unsim = S - P
        npos = cnt - 1.0
        loss = (npos * np.log(unsim) + (P - E5) / unsim
                - INV_T * bbv + INV_T)
        total += float(loss.sum())
    return np.array(total / n_pos, dtype=np.float32)


LAST_RESULTS = None


# revision 5
# speedup vs baseline: 1.4548x; 1.1076x over previous
"""Trainium2 Bass kernel for nn_ContrastiveLoss (NT-Xent style contrastive loss).

Strategy (8 NeuronCores, SPMD):
  - Host sorts samples by label (the scalar loss is permutation invariant),
    row-normalizes, and builds X^T [D=128, N=8192] in bf16.
  - Rows are sharded across 8 cores (1024 rows each, 8 blocks of 128).
  - Each core computes its [1024, 8192] similarity block against the full
    X^T; the exp row-sums are split between the ACT engine (exact Exp with
    accum) and the DVE (fp16 Schraudolph bit-trick exp + reduce), balancing
    the two engine's throughputs. The positive-pair window (sorted labels
    make positives contiguous) is handled by a masked band: one fused DVE
    affine_mul_reduce gives both the masked sims (exp input) and their sum B;
    one ACT Exp-with-accum gives the masked exp sum.
  - Device outputs per-row partial sums only (no log/reciprocal on device:
    the ACT engine keeps a single Exp table load, no table thrash). The host
    computes the final scalar:
      loss_row = npos*log(unsim) + (P - e^5)/unsim - 5*B + 5
    using log1p(x) ~= x (x = e_ij/unsim <= 1.6e-3 here; error ~1e-7 rel).
  - The Schraudolph share's multiplicative bias is calibrated out on the
    host from a small sampled subset of the exact sims (divide by rbar).
"""

import numpy as np

T = 0.2
INV_T = 1.0 / T  # 5.0
EPS = 1e-5
N, D, NCLASS = 8192, 128, 128
NCORES = 8
ROWS_PER_CORE = N // NCORES          # 1024
BLOCKS = ROWS_PER_CORE // 128        # 8 blocks of 128 rows per core
CHUNK = 2048                         # PSUM chunk (4 banks)
NCHUNKS = N // CHUNK                 # 4 per block
MM = 512                             # matmul free-dim per PSUM bank
AW = 1536                            # ACT's columns per chunk (exact Exp)
DW = CHUNK - AW                      # DVE's columns per chunk (Schraudolph)

# fp16 Schraudolph: exp(5*s) ~= bitcast_i16_to_f16(round(s*SCH_A + SCH_B))
SCH_A = INV_T * 1024.0 * 1.4426950408889634   # 5 * 1024 * log2(e)
SCH_B = 15302.0                               # 15*1024 - 58 (mean-centering)

_CACHE = {}


def _build_nc(W, debug=False):
    """Build the SPMD Bass/Tile program. W = band window width (mult of 512)."""
    import concourse.bass as bass
    import concourse.bacc as bacc
    import concourse.mybir as mybir
    import concourse.tile as tile

    dt = mybir.dt
    AF = mybir.ActivationFunctionType
    ALU = mybir.AluOpType
    X = mybir.AxisListType.X

    nc = bacc.Bacc("TRN2", target_bir_lowering=False, debug=debug)

    xt_d = nc.dram_tensor("xt", [128, N], dt.bfloat16, kind="ExternalInput")
    xtown_d = nc.dram_tensor("xtown", [128, ROWS_PER_CORE], dt.bfloat16,
                             kind="ExternalInput")
    xtband_d = nc.dram_tensor("xtband", [128, BLOCKS * W], dt.bfloat16,
                              kind="ExternalInput")
    gsr_d = nc.dram_tensor("gsr", [128, BLOCKS], dt.float32, kind="ExternalInput")
    ger_d = nc.dram_tensor("ger", [128, BLOCKS], dt.float32, kind="ExternalInput")
    sp_d = nc.dram_tensor("sp", [128, BLOCKS * NCHUNKS], dt.float32,
                          kind="ExternalOutput")
    dp_d = nc.dram_tensor("dp", [128, BLOCKS * NCHUNKS], dt.float16,
                          kind="ExternalOutput")
    a1_d = nc.dram_tensor("a1", [128, BLOCKS], dt.float32, kind="ExternalOutput")
    bb_d = nc.dram_tensor("bb", [128, BLOCKS], dt.float32, kind="ExternalOutput")

    nwc = W // MM  # band matmul sub-chunks

    with tile.TileContext(nc) as tc:
        with (
            tc.tile_pool(name="const", bufs=1) as const,
            tc.tile_pool(name="escr", bufs=3) as escr_pool,
            tc.tile_pool(name="i16", bufs=3) as i16_pool,
            tc.tile_pool(name="ms", bufs=2) as ms_pool,
            tc.tile_pool(name="psum", bufs=2, space="PSUM") as psum,
        ):
            # ---- small inputs first (masks depend on them) ----
            gsr = const.tile([128, BLOCKS], dt.float32)
            nc.sync.dma_start(gsr[:], gsr_d[:])
            ger = const.tile([128, BLOCKS], dt.float32)
            nc.sync.dma_start(ger[:], ger_d[:])
            xtown = const.tile([128, ROWS_PER_CORE], dt.bfloat16)
            nc.sync.dma_start(xtown[:], xtown_d[:])
            xt = const.tile([128, N], dt.bfloat16)
            for k in range(N // 1024):
                nc.sync.dma_start(xt[:, k * 1024:(k + 1) * 1024],
                                  xt_d[:, k * 1024:(k + 1) * 1024])
            xtband = const.tile([128, BLOCKS * W], dt.bfloat16)
            nc.sync.dma_start(xtband[:], xtband_d[:])

            iota_i = const.tile([128, W], dt.int32)
            nc.gpsimd.iota(iota_i[:], pattern=[[1, W]], base=0,
                           channel_multiplier=0)
            iota_f = const.tile([128, W], dt.float32)
            nc.vector.tensor_copy(iota_f[:], iota_i[:])

            # ---- persistent accumulator outputs ----
            sparts = const.tile([128, BLOCKS * NCHUNKS], dt.float32)
            dparts = const.tile([128, BLOCKS * NCHUNKS], dt.float16)
            a1 = const.tile([128, BLOCKS], dt.float32)
            bb = const.tile([128, BLOCKS], dt.float32)
            masks = [const.tile([128, W], dt.float32, name=f"mk{b}")
                     for b in range(BLOCKS)]

            for b in range(BLOCKS):
                # band mask for this block (DVE, overlaps with dense below)
                m1 = ms_pool.tile([128, W], dt.float32, tag="m1")
                nc.vector.tensor_scalar(m1[:], iota_f[:], gsr[:, b:b + 1],
                                        None, op0=ALU.is_ge)
                nc.vector.scalar_tensor_tensor(masks[b][:], iota_f[:],
                                               ger[:, b:b + 1], m1[:],
                                               op0=ALU.is_lt, op1=ALU.mult)

                lhsT = xtown[:, b * 128:(b + 1) * 128]
                for kc in range(NCHUNKS):
                    ps = psum.tile([128, CHUNK], dt.float32, tag="ps")
                    for j in range(CHUNK // MM):
                        c0 = kc * CHUNK + j * MM
                        nc.tensor.matmul(ps[:, j * MM:(j + 1) * MM], lhsT,
                                         xt[:, c0:c0 + MM], start=True,
                                         stop=True)
                    col = b * NCHUNKS + kc
                    # ACT: exact exp + row-sum accumulate on [0:AW]
                    e_scr = escr_pool.tile([128, AW], dt.bfloat16, tag="et")
                    nc.scalar.activation(e_scr[:], ps[:, 0:AW], AF.Exp,
                                         bias=0.0, scale=INV_T,
                                         accum_out=sparts[:, col:col + 1])
                    # DVE: Schraudolph fp16 exp + reduce on [AW:CHUNK]
                    i16 = i16_pool.tile([128, DW], dt.int16, tag="i16")
                    nc.vector.tensor_scalar(i16[:], ps[:, AW:CHUNK],
                                            SCH_A, SCH_B,
                                            op0=ALU.mult, op1=ALU.add)
                    with nc.allow_low_precision("fp16 partial; +-0.25 on ~1e3, corrected host-side"):
                        nc.vector.reduce_sum(dparts[:, col:col + 1],
                                             i16[:].bitcast(dt.float16),
                                             axis=X)

                # band: masked sims (one fused DVE op -> ms and B), then
                # one ACT exp with accum -> masked exp sum (+ W-count ones)
                psb = psum.tile([128, W], dt.float32, tag="ps")
                nc.tensor.matmul(psb[:], lhsT, xtband[:, b * W:(b + 1) * W],
                                 start=True, stop=True)
                ms = ms_pool.tile([128, W], dt.float32, tag="ms")
                nc.vector.affine_mul_reduce(ms[:], bb[:, b:b + 1],
                                            psb[:], masks[b][:], 1.0, 0.0)
                eb = escr_pool.tile([128, W], dt.bfloat16, tag="eb")
                nc.scalar.activation(eb[:], ms[:], AF.Exp, bias=0.0,
                                     scale=INV_T,
                                     accum_out=a1[:, b:b + 1])

            nc.sync.dma_start(sp_d[:], sparts[:])
            nc.sync.dma_start(dp_d[:], dparts[:])
            nc.sync.dma_start(a1_d[:], a1[:])
            nc.sync.dma_start(bb_d[:], bb[:])

    nc.compile()
    return nc


def _prep(input, label):
    """Host-side shard prep: sort by label, normalize, build per-core inputs."""
    import ml_dtypes

    x = np.asarray(input, dtype=np.float32).reshape(N, D)
    lab = np.asarray(label).astype(np.int64).reshape(N)

    order = np.argsort(lab, kind="stable")
    xs, ls = x[order], lab[order]
    counts = np.bincount(ls, minlength=NCLASS)
    n_pos = int((counts.astype(np.int64) ** 2).sum()) - N
    ends = np.cumsum(counts)
    starts = ends - counts
    row_gs = starts[ls]          # [N] group start col per (sorted) row
    row_ge = ends[ls]            # [N] group end col per row

    norms = np.sqrt((xs * xs).sum(1, dtype=np.float32)).astype(np.float32)
    # reference divides by max(n_i*n_j, EPS); for this data the max never
    # binds (norms ~ 11), so plain normalization is exact.
    assert float(norms.min()) ** 2 > EPS * 1.0001
    xn = (xs / norms[:, None]).astype(np.float32)
    xt = np.ascontiguousarray(xn.T).astype(ml_dtypes.bfloat16)  # [128, N]

    # Schraudolph bias calibration on a sampled subset of exact sims
    xnb = xt.T.astype(np.float32)                       # bf16-rounded rows
    samp = xnb[:: N // 48] @ xnb.T                      # [~48, N] sims
    ys = np.float64(INV_T) * samp.astype(np.float64)
    i16 = np.rint(samp * SCH_A + SCH_B).astype(np.int16)
    rbar = float(i16.view(np.float16).astype(np.float64).sum()
                 / np.exp(ys).sum())

    # band windows per global block
    nblk = N // 128
    lo = row_gs[np.arange(nblk) * 128]
    hi = row_ge[np.arange(nblk) * 128 + 127]
    maxband = int((hi - lo).max())
    W = max(512, ((maxband + 511) // 512) * 512)
    wstart = np.minimum(lo, N - W)

    in_maps = []
    for c in range(NCORES):
        r0 = c * ROWS_PER_CORE
        xtband = np.empty((128, BLOCKS * W), dtype=ml_dtypes.bfloat16)
        gsr = np.empty((128, BLOCKS), np.float32)
        ger = np.empty((128, BLOCKS), np.float32)
        for b in range(BLOCKS):
            g = c * BLOCKS + b
            ws = int(wstart[g])
            xtband[:, b * W:(b + 1) * W] = xt[:, ws:ws + W]
            rows = slice(r0 + b * 128, r0 + (b + 1) * 128)
            gsr[:, b] = (row_gs[rows] - ws).astype(np.float32)
            ger[:, b] = (row_ge[rows] - ws).astype(np.float32)
        in_maps.append({
            "xt": xt,
            "xtown": np.ascontiguousarray(xt[:, r0:r0 + ROWS_PER_CORE]),
            "xtband": xtband,
            "gsr": gsr,
            "ger": ger,
        })
    counts_row = (row_ge - row_gs).astype(np.float64)   # class size per row
    return in_maps, n_pos, W, counts_row, rbar


def kernel(input, label):
    from concourse.bass_utils import run_bass_kernel_spmd

    in_maps, n_pos, W, counts_row, rbar = _prep(input, label)
    if W not in _CACHE:
        _CACHE[W] = _build_nc(W)
    nc = _CACHE[W]

    res = None
    for attempt in range(4):
        try:
            res = run_bass_kernel_spmd(nc, in_maps, core_ids=list(range(NCORES)))
            break
        except Exception:
            if attempt == 3:
                raise
            import time
            time.sleep(45)  # device may need a moment to recover
    global LAST_RESULTS
    LAST_RESULTS = res

    E5 = float(np.exp(5.0))
    total = 0.0
    for c, r in enumerate(res.results):
        sp = np.asarray(r["sp"], dtype=np.float64)      # [128, BLOCKS*NCHUNKS]
        dp = np.asarray(r["dp"], dtype=np.float64)
        a1 = np.asarray(r["a1"], dtype=np.float64)      # [128, BLOCKS]
        bbv = np.asarray(r["bb"], dtype=np.float64)
        # S per row: exact ACT part + bias-corrected Schraudolph part
        S = (sp.reshape(128, BLOCKS, NCHUNKS).sum(axis=2)
             + dp.reshape(128, BLOCKS, NCHUNKS).sum(axis=2) / rbar)  # [128, B]
        cnt = counts_row[c * ROWS_PER_CORE:(c + 1) * ROWS_PER_CORE]
        cnt = cnt.reshape(BLOCKS, 128).T                # [128, BLOCKS]
        P = a1 - (W - cnt)                              # masked exp sum w/ diag
        unsim = S - P
        npos = cnt - 1.0
        loss = (npos * np.log(unsim) + (P - E5) / unsim
                - INV_T * bbv + INV_T)
        total += float(loss.sum())
    return np.array(total / n_pos, dtype=np.float32)


LAST_RESULTS = None
